# revision 20
# baseline (speedup 1.0000x reference)
"""Trainium2 Bass kernel for nn_ConditionedDense (hypernetwork-conditioned dense).

Reference computation:
    A = einsum('bnp,pq->bnq', P, Wk)         # hypernetwork: per-position weights
    W = relu(A).reshape(B, N, c_in, c_out)
    out = einsum('bni,bnio->bno', X, W)

Strategy: pure data parallel over 8 NeuronCores (shard batch dim), 16384
positions/core, layout [pos, q] with q = o*32+i, 16 chunks of 1024
positions (8 tiles of 128).  Per chunk:
  - PE matmul fills A tiles [128 pos, 1024 q] in PSUM (lhsT = P^T tile,
    rhs = Wk, bf16).
  - ACT evacuates PSUM -> SBUF bf16 with fused relu (DVE takes a few
    units to balance).
  - The X-multiply is split 5:3 between DVE (tensor_tensor, 2x bf16) and
    GPSIMD (slow 0.42-efficiency Q7 engine, but otherwise idle).
  - DVE reduces over i (innermost 32) with a 5-stage halving tree (all
    2x tensor_tensor; the last stride-2 stage runs 1x but is tiny —
    cheaper than baseline's tensor_reduce ending).
Chunk trees and output DMAs are emitted one chunk late ("post" queue) so
no in-order engine queue ever head-of-line blocks on the slow GPSIMD
multiply.  Engine budget/core: DVE ~112us, ACT ~120, GPSIMD ~97, PE ~119
(baseline: DVE 151, ACT 124, GPSIMD 0 -> 177us wall).
Host side (free): shard, transpose P, permute Wk columns, cast to bf16.
"""

import os
from contextlib import ExitStack

import numpy as np
import ml_dtypes

import concourse.bass as bass
import concourse.tile as tile
from concourse import bacc, mybir
from concourse.bass_utils import run_bass_kernel_spmd

C_IN = 32
C_OUT = 32
P_DIM = 64
Q = C_IN * C_OUT  # 1024
B, N = 32, 4096
N_CORES = 8
B_SH = B // N_CORES          # 4 batches per core
NPOS = B_SH * N              # 16384 positions per core
TILE_P = 128                 # positions per tile
N_TILES = NPOS // TILE_P     # 128
CHUNK = 8                    # tiles per DMA chunk
N_CHUNKS = N_TILES // CHUNK  # 16

F32 = mybir.dt.float32
BF16 = mybir.dt.bfloat16

_BUILD_CACHE = {}
LAST_RESULTS = None  # BassKernelResults of the most recent run (for profiling)

POOL_MUL_TILES = 3   # of the 8 tiles per chunk, how many multiply on GPSIMD
DVE_EVAC = {2, 9}    # evac unit indices (of 64) routed to DVE instead of ACT


def _build_nc():
    nc = bacc.Bacc(
        "TRN2", target_bir_lowering=False, debug=False, num_devices=N_CORES
    )
    X_d = nc.declare_dram_parameter("X", [NPOS, C_IN], BF16, isOutput=False)
    PT_d = nc.declare_dram_parameter("PT", [P_DIM, NPOS], BF16, isOutput=False)
    Wk_d = nc.declare_dram_parameter("Wk", [P_DIM, Q], BF16, isOutput=False)
    out_d = nc.declare_dram_parameter("out", [NPOS, C_OUT], BF16, isOutput=True)

    relu = mybir.ActivationFunctionType.Relu
    mult = mybir.AluOpType.mult
    add = mybir.AluOpType.add

    ev_idx = [0]

    with ExitStack() as ctx:
        tc = ctx.enter_context(tile.TileContext(nc))
        wkp = ctx.enter_context(tc.tile_pool(name="wk", bufs=1))
        xp = ctx.enter_context(tc.tile_pool(name="x", bufs=3))
        pp = ctx.enter_context(tc.tile_pool(name="pT", bufs=3))
        apool = ctx.enter_context(tc.tile_pool(name="apsum", bufs=2, space="PSUM"))
        wp = ctx.enter_context(tc.tile_pool(name="w", bufs=3))
        mp = ctx.enter_context(tc.tile_pool(name="m", bufs=3))
        t1p = ctx.enter_context(tc.tile_pool(name="t1", bufs=2))
        t2p = ctx.enter_context(tc.tile_pool(name="t2", bufs=2))
        t3p = ctx.enter_context(tc.tile_pool(name="t3", bufs=2))
        t4p = ctx.enter_context(tc.tile_pool(name="t4", bufs=2))
        op = ctx.enter_context(tc.tile_pool(name="o", bufs=3))

        wk_t = wkp.tile([P_DIM, Q], BF16)
        nc.sync.dma_start(out=wk_t[:], in_=Wk_d[:])

        def evac(dst, src):
            """PSUM f32 -> SBUF bf16 with relu, mostly ACT."""
            if ev_idx[0] % (N_TILES // 2) in DVE_EVAC:
                nc.vector.tensor_scalar_max(dst, src, 0.0)
            else:
                nc.scalar.activation(dst, src, relu)
            ev_idx[0] += 1

        post = []        # [(due_chunk, closure)] deferred emissions
        step = [0]

        def flush_post():
            rest = []
            for due, f in post:
                if due <= step[0]:
                    f()
                else:
                    rest.append((due, f))
            post[:] = rest

        PAIR = 2   # tiles per PSUM tile (4 banks; 2 PSUM tiles in flight)
        DSPLIT = CHUNK - POOL_MUL_TILES  # tiles multiplied on DVE
        for ch in range(N_CHUNKS):
            step[0] = ch
            flush_post()
            x_c = xp.tile([TILE_P, CHUNK, C_IN], BF16)
            nc.sync.dma_start(
                out=x_c[:],
                in_=X_d[bass.ts(ch, TILE_P * CHUNK), :].rearrange(
                    "(a p) i -> p a i", p=TILE_P
                ),
            )
            pT_c = pp.tile([P_DIM, CHUNK * TILE_P], BF16)
            nc.sync.dma_start(
                out=pT_c[:], in_=PT_d[:, bass.ts(ch, TILE_P * CHUNK)]
            )

            w_c = wp.tile([TILE_P, CHUNK, Q], BF16)
            for h in range(CHUNK // PAIR):
                a_t = apool.tile([TILE_P, PAIR, Q], F32)
                for j in range(PAIR):
                    lhsT = pT_c[:, bass.ts(h * PAIR + j, TILE_P)]
                    nc.tensor.matmul(
                        a_t[:, j, 0:512], lhsT=lhsT, rhs=wk_t[:, 0:512],
                        start=True, stop=True,
                    )
                    nc.tensor.matmul(
                        a_t[:, j, 512:1024], lhsT=lhsT,
                        rhs=wk_t[:, 512:1024], start=True, stop=True,
                    )
                evac(w_c[:, bass.ts(h, PAIR), :], a_t[:])

            # m[p, j, o, i] = w[p, j, o, i] * x[p, j, i]: DVE takes the
            # first DSPLIT tiles (2x bf16), GPSIMD the rest.
            m_t = mp.tile([TILE_P, CHUNK, Q], BF16)
            w4 = w_c[:].rearrange("p j (o i) -> p j o i", o=C_OUT)
            m4 = m_t[:].rearrange("p j (o i) -> p j o i", o=C_OUT)
            x4 = x_c[:].unsqueeze(2).broadcast_to(
                [TILE_P, CHUNK, C_OUT, C_IN]
            )
            nc.vector.tensor_tensor(
                out=m4[:, 0:DSPLIT], in0=w4[:, 0:DSPLIT],
                in1=x4[:, 0:DSPLIT], op=mult,
            )
            nc.gpsimd.tensor_tensor(
                out=m4[:, DSPLIT:CHUNK], in0=w4[:, DSPLIT:CHUNK],
                in1=x4[:, DSPLIT:CHUNK], op=mult,
            )

            def emit_tree(m4=m4, ch=ch):
                # 5-stage halving tree over i (innermost 32), all on DVE
                t1 = t1p.tile([TILE_P, CHUNK, C_OUT, 16], BF16)
                nc.vector.tensor_tensor(
                    out=t1[:], in0=m4[:, :, :, 0:16], in1=m4[:, :, :, 16:32],
                    op=add,
                )
                t2 = t2p.tile([TILE_P, CHUNK, C_OUT, 8], BF16)
                nc.vector.tensor_tensor(
                    out=t2[:], in0=t1[:, :, :, 0:8], in1=t1[:, :, :, 8:16],
                    op=add,
                )
                t3 = t3p.tile([TILE_P, CHUNK, C_OUT, 4], BF16)
                nc.vector.tensor_tensor(
                    out=t3[:], in0=t2[:, :, :, 0:4], in1=t2[:, :, :, 4:8],
                    op=add,
                )
                t4 = t4p.tile([TILE_P, CHUNK, C_OUT, 2], BF16)
                nc.vector.tensor_tensor(
                    out=t4[:], in0=t3[:, :, :, 0:2], in1=t3[:, :, :, 2:4],
                    op=add,
                )
                o_c = op.tile([TILE_P, CHUNK, C_OUT], BF16)
                nc.vector.tensor_tensor(
                    out=o_c[:].unsqueeze(3), in0=t4[:, :, :, 0:1],
                    in1=t4[:, :, :, 1:2], op=add,
                )
                post.append((step[0] + 1, lambda: nc.sync.dma_start(
                    out=out_d[bass.ts(ch, TILE_P * CHUNK), :].rearrange(
                        "(a p) i -> p a i", p=TILE_P
                    ),
                    in_=o_c[:],
                )))

            # defer the tree one chunk so DVE's queue never waits on the
            # slow GPSIMD multiply
            post.append((ch + 1, emit_tree))

        while post:
            step[0] += 1
            flush_post()

    nc.finalize()
    return nc


def _get_nc():
    key = "v10"
    if key not in _BUILD_CACHE:
        _BUILD_CACHE[key] = _build_nc()
    return _BUILD_CACHE[key]


def kernel(X, P, Wk):
    global LAST_RESULTS
    X = np.asarray(X, dtype=np.float32)
    P = np.asarray(P, dtype=np.float32)
    Wk = np.asarray(Wk, dtype=np.float32)
    bf16 = ml_dtypes.bfloat16

    # Host-side prep (free): shard, transpose P, permute Wk columns so the
    # device-side layout is q = o*32+i; cast matmul operands to bf16.
    WkP = np.ascontiguousarray(
        Wk.reshape(P_DIM, C_IN, C_OUT).transpose(0, 2, 1).reshape(P_DIM, Q)
    ).astype(bf16)
    in_maps = []
    for c in range(N_CORES):
        Xc = np.ascontiguousarray(
            X[c * B_SH:(c + 1) * B_SH].reshape(NPOS, C_IN)
        ).astype(bf16)
        PTc = np.ascontiguousarray(
            P[c * B_SH:(c + 1) * B_SH].reshape(NPOS, P_DIM).T
        ).astype(bf16)
        in_maps.append({"X": Xc, "PT": PTc, "Wk": WkP})

    nc = _get_nc()
    trace = os.environ.get("BASS_PROFILE", "0") == "1"
    kw = {}
    if os.environ.get("BASS_TMPDIR"):
        kw["tmpdir"] = os.environ["BASS_TMPDIR"]
    res = run_bass_kernel_spmd(
        nc, in_maps, list(range(N_CORES)), trace=trace, **kw
    )
    LAST_RESULTS = res

    out = np.empty((B, N, C_OUT), dtype=np.float32)
    for c in range(N_CORES):
        out[c * B_SH:(c + 1) * B_SH] = (
            np.asarray(res.results[c]["out"])
            .astype(np.float32)
            .reshape(B_SH, N, C_OUT)
        )
    return out


# revision 21
# speedup vs baseline: 1.6800x; 1.6800x over previous
"""Trainium2 Bass kernel for nn_ConditionedDense (hypernetwork-conditioned dense).

Reference computation:
    A = einsum('bnp,pq->bnq', P, Wk)         # hypernetwork: per-position weights
    W = relu(A).reshape(B, N, c_in, c_out)
    out = einsum('bni,bnio->bno', X, W)

Strategy: pure data parallel over 8 NeuronCores (shard batch dim), 16384
positions/core, layout [pos, q] with q = o*32+i, 16 chunks of 1024
positions (8 tiles of 128).  Per chunk:
  - PE matmul fills A tiles [128 pos, 1024 q] in PSUM (lhsT = P^T tile,
    rhs = Wk, bf16).
  - ACT evacuates PSUM -> SBUF bf16 with fused relu (DVE takes a few
    units to balance).
  - The X-multiply is split 5:3 between DVE (tensor_tensor, 2x bf16) and
    GPSIMD (slow 0.42-efficiency Q7 engine, but otherwise idle).
  - DVE reduces over i (innermost 32) with a 5-stage halving tree (all
    2x tensor_tensor; the last stride-2 stage runs 1x but is tiny —
    cheaper than baseline's tensor_reduce ending).
Chunk trees and output DMAs are emitted one chunk late ("post" queue) so
no in-order engine queue ever head-of-line blocks on the slow GPSIMD
multiply.  Engine budget/core: DVE ~112us, ACT ~120, GPSIMD ~97, PE ~119
(baseline: DVE 151, ACT 124, GPSIMD 0 -> 177us wall).
Host side (free): shard, transpose P, permute Wk columns, cast to bf16.
"""

import os
from contextlib import ExitStack

import numpy as np
import ml_dtypes

import concourse.bass as bass
import concourse.tile as tile
from concourse import bacc, mybir
from concourse.bass_utils import run_bass_kernel_spmd

C_IN = 32
C_OUT = 32
P_DIM = 64
Q = C_IN * C_OUT  # 1024
B, N = 32, 4096
N_CORES = 8
B_SH = B // N_CORES          # 4 batches per core
NPOS = B_SH * N              # 16384 positions per core
TILE_P = 128                 # positions per tile
N_TILES = NPOS // TILE_P     # 128
CHUNK = 8                    # tiles per DMA chunk
N_CHUNKS = N_TILES // CHUNK  # 16

F32 = mybir.dt.float32
BF16 = mybir.dt.bfloat16

_BUILD_CACHE = {}
LAST_RESULTS = None  # BassKernelResults of the most recent run (for profiling)

POOL_MUL_TILES = 0   # of the 8 tiles per chunk, how many multiply on GPSIMD
DVE_EVAC = set()     # evac unit indices (of 64) routed to DVE instead of ACT


def _build_nc():
    nc = bacc.Bacc(
        "TRN2", target_bir_lowering=False, debug=False, num_devices=N_CORES
    )
    X_d = nc.declare_dram_parameter("X", [NPOS, C_IN], BF16, isOutput=False)
    PT_d = nc.declare_dram_parameter("PT", [P_DIM, NPOS], BF16, isOutput=False)
    Wk_d = nc.declare_dram_parameter("Wk", [P_DIM, Q], BF16, isOutput=False)
    out_d = nc.declare_dram_parameter("out", [NPOS, C_OUT], BF16, isOutput=True)

    relu = mybir.ActivationFunctionType.Relu
    mult = mybir.AluOpType.mult
    add = mybir.AluOpType.add

    ev_idx = [0]

    with ExitStack() as ctx:
        tc = ctx.enter_context(tile.TileContext(nc))
        wkp = ctx.enter_context(tc.tile_pool(name="wk", bufs=1))
        xp = ctx.enter_context(tc.tile_pool(name="x", bufs=3))
        pp = ctx.enter_context(tc.tile_pool(name="pT", bufs=3))
        apool = ctx.enter_context(tc.tile_pool(name="apsum", bufs=2, space="PSUM"))
        wp = ctx.enter_context(tc.tile_pool(name="w", bufs=3))
        mp = ctx.enter_context(tc.tile_pool(name="m", bufs=3))
        t1p = ctx.enter_context(tc.tile_pool(name="t1", bufs=2))
        t2p = ctx.enter_context(tc.tile_pool(name="t2", bufs=2))
        t3p = ctx.enter_context(tc.tile_pool(name="t3", bufs=2))
        t4p = ctx.enter_context(tc.tile_pool(name="t4", bufs=2))
        op = ctx.enter_context(tc.tile_pool(name="o", bufs=3))

        wk_t = wkp.tile([P_DIM, Q], BF16)
        nc.sync.dma_start(out=wk_t[:], in_=Wk_d[:])

        def evac(dst, src):
            """PSUM f32 -> SBUF bf16 with relu, mostly ACT."""
            if ev_idx[0] % (N_TILES // 2) in DVE_EVAC:
                nc.vector.tensor_scalar_max(dst, src, 0.0)
            else:
                nc.scalar.activation(dst, src, relu)
            ev_idx[0] += 1

        post = []        # [(due_chunk, closure)] deferred emissions
        step = [0]

        def flush_post():
            rest = []
            for due, f in post:
                if due <= step[0]:
                    f()
                else:
                    rest.append((due, f))
            post[:] = rest

        PAIR = 2   # tiles per PSUM tile (4 banks; 2 PSUM tiles in flight)
        DSPLIT = CHUNK - POOL_MUL_TILES  # tiles multiplied on DVE
        for ch in range(N_CHUNKS):
            step[0] = ch
            flush_post()
            x_c = xp.tile([TILE_P, CHUNK, C_IN], BF16)
            nc.sync.dma_start(
                out=x_c[:],
                in_=X_d[bass.ts(ch, TILE_P * CHUNK), :].rearrange(
                    "(a p) i -> p a i", p=TILE_P
                ),
            )
            pT_c = pp.tile([P_DIM, CHUNK * TILE_P], BF16)
            nc.sync.dma_start(
                out=pT_c[:], in_=PT_d[:, bass.ts(ch, TILE_P * CHUNK)]
            )

            w_c = wp.tile([TILE_P, CHUNK, Q], BF16)
            for h in range(CHUNK // PAIR):
                a_t = apool.tile([TILE_P, PAIR, Q], F32)
                for j in range(PAIR):
                    lhsT = pT_c[:, bass.ts(h * PAIR + j, TILE_P)]
                    nc.tensor.matmul(
                        a_t[:, j, 0:512], lhsT=lhsT, rhs=wk_t[:, 0:512],
                        start=True, stop=True,
                    )
                    nc.tensor.matmul(
                        a_t[:, j, 512:1024], lhsT=lhsT,
                        rhs=wk_t[:, 512:1024], start=True, stop=True,
                    )
                evac(w_c[:, bass.ts(h, PAIR), :], a_t[:])

            # m[p, j, o, i] = w[p, j, o, i] * x[p, j, i]: DVE takes the
            # first DSPLIT tiles (2x bf16), GPSIMD the rest.
            m_t = mp.tile([TILE_P, CHUNK, Q], BF16)
            w4 = w_c[:].rearrange("p j (o i) -> p j o i", o=C_OUT)
            m4 = m_t[:].rearrange("p j (o i) -> p j o i", o=C_OUT)
            x4 = x_c[:].unsqueeze(2).broadcast_to(
                [TILE_P, CHUNK, C_OUT, C_IN]
            )
            nc.vector.tensor_tensor(
                out=m4[:, 0:DSPLIT], in0=w4[:, 0:DSPLIT],
                in1=x4[:, 0:DSPLIT], op=mult,
            )
            if DSPLIT < CHUNK:
                nc.gpsimd.tensor_tensor(
                    out=m4[:, DSPLIT:CHUNK], in0=w4[:, DSPLIT:CHUNK],
                    in1=x4[:, DSPLIT:CHUNK], op=mult,
                )

            def emit_tree(m4=m4, ch=ch):
                # 5-stage halving tree over i (innermost 32), all on DVE
                t1 = t1p.tile([TILE_P, CHUNK, C_OUT, 16], BF16)
                nc.vector.tensor_tensor(
                    out=t1[:], in0=m4[:, :, :, 0:16], in1=m4[:, :, :, 16:32],
                    op=add,
                )
                t2 = t2p.tile([TILE_P, CHUNK, C_OUT, 8], BF16)
                nc.vector.tensor_tensor(
                    out=t2[:], in0=t1[:, :, :, 0:8], in1=t1[:, :, :, 8:16],
                    op=add,
                )
                t3 = t3p.tile([TILE_P, CHUNK, C_OUT, 4], BF16)
                nc.vector.tensor_tensor(
                    out=t3[:], in0=t2[:, :, :, 0:4], in1=t2[:, :, :, 4:8],
                    op=add,
                )
                t4 = t4p.tile([TILE_P, CHUNK, C_OUT, 2], BF16)
                nc.vector.tensor_tensor(
                    out=t4[:], in0=t3[:, :, :, 0:2], in1=t3[:, :, :, 2:4],
                    op=add,
                )
                o_c = op.tile([TILE_P, CHUNK, C_OUT], BF16)
                nc.vector.tensor_tensor(
                    out=o_c[:].unsqueeze(3), in0=t4[:, :, :, 0:1],
                    in1=t4[:, :, :, 1:2], op=add,
                )
                post.append((step[0] + 1, lambda: nc.sync.dma_start(
                    out=out_d[bass.ts(ch, TILE_P * CHUNK), :].rearrange(
                        "(a p) i -> p a i", p=TILE_P
                    ),
                    in_=o_c[:],
                )))

            # defer the tree one chunk so DVE's queue never waits on the
            # slow GPSIMD multiply
            post.append((ch + 1, emit_tree))

        while post:
            step[0] += 1
            flush_post()

    nc.finalize()
    return nc


def _get_nc():
    key = "v11"
    if key not in _BUILD_CACHE:
        _BUILD_CACHE[key] = _build_nc()
    return _BUILD_CACHE[key]


def kernel(X, P, Wk):
    global LAST_RESULTS
    X = np.asarray(X, dtype=np.float32)
    P = np.asarray(P, dtype=np.float32)
    Wk = np.asarray(Wk, dtype=np.float32)
    bf16 = ml_dtypes.bfloat16

    # Host-side prep (free): shard, transpose P, permute Wk columns so the
    # device-side layout is q = o*32+i; cast matmul operands to bf16.
    WkP = np.ascontiguousarray(
        Wk.reshape(P_DIM, C_IN, C_OUT).transpose(0, 2, 1).reshape(P_DIM, Q)
    ).astype(bf16)
    in_maps = []
    for c in range(N_CORES):
        Xc = np.ascontiguousarray(
            X[c * B_SH:(c + 1) * B_SH].reshape(NPOS, C_IN)
        ).astype(bf16)
        PTc = np.ascontiguousarray(
            P[c * B_SH:(c + 1) * B_SH].reshape(NPOS, P_DIM).T
        ).astype(bf16)
        in_maps.append({"X": Xc, "PT": PTc, "Wk": WkP})

    nc = _get_nc()
    trace = os.environ.get("BASS_PROFILE", "0") == "1"
    kw = {}
    if os.environ.get("BASS_TMPDIR"):
        kw["tmpdir"] = os.environ["BASS_TMPDIR"]
    res = run_bass_kernel_spmd(
        nc, in_maps, list(range(N_CORES)), trace=trace, **kw
    )
    LAST_RESULTS = res

    out = np.empty((B, N, C_OUT), dtype=np.float32)
    for c in range(N_CORES):
        out[c * B_SH:(c + 1) * B_SH] = (
            np.asarray(res.results[c]["out"])
            .astype(np.float32)
            .reshape(B_SH, N, C_OUT)
        )
    return out


# revision 22
# speedup vs baseline: 1.6956x; 1.0093x over previous
"""Trainium2 Bass kernel for nn_ConditionedDense (hypernetwork-conditioned dense).

Reference computation:
    A = einsum('bnp,pq->bnq', P, Wk)         # hypernetwork: per-position weights
    W = relu(A).reshape(B, N, c_in, c_out)
    out = einsum('bni,bnio->bno', X, W)

Strategy: pure data parallel over 8 NeuronCores (shard batch dim), 16384
positions/core, layout [pos, q] with q = o*32+i, 16 chunks of 1024
positions (8 tiles of 128).  Per chunk:
  - PE matmul fills A tiles [128 pos, 1024 q] in PSUM (lhsT = P^T tile,
    rhs = Wk, bf16).
  - ACT evacuates PSUM -> SBUF bf16 with fused relu (DVE takes a few
    units to balance).
  - The X-multiply is split 5:3 between DVE (tensor_tensor, 2x bf16) and
    GPSIMD (slow 0.42-efficiency Q7 engine, but otherwise idle).
  - DVE reduces over i (innermost 32) with a 5-stage halving tree (all
    2x tensor_tensor; the last stride-2 stage runs 1x but is tiny —
    cheaper than baseline's tensor_reduce ending).
Chunk trees and output DMAs are emitted one chunk late ("post" queue) so
no in-order engine queue ever head-of-line blocks on the slow GPSIMD
multiply.  Engine budget/core: DVE ~112us, ACT ~120, GPSIMD ~97, PE ~119
(baseline: DVE 151, ACT 124, GPSIMD 0 -> 177us wall).
Host side (free): shard, transpose P, permute Wk columns, cast to bf16.
"""

import os
from contextlib import ExitStack

import numpy as np
import ml_dtypes

import concourse.bass as bass
import concourse.tile as tile
from concourse import bacc, mybir
from concourse.bass_utils import run_bass_kernel_spmd

C_IN = 32
C_OUT = 32
P_DIM = 64
Q = C_IN * C_OUT  # 1024
B, N = 32, 4096
N_CORES = 8
B_SH = B // N_CORES          # 4 batches per core
NPOS = B_SH * N              # 16384 positions per core
TILE_P = 128                 # positions per tile
N_TILES = NPOS // TILE_P     # 128
CHUNK = 8                    # tiles per DMA chunk
N_CHUNKS = N_TILES // CHUNK  # 16

F32 = mybir.dt.float32
BF16 = mybir.dt.bfloat16

_BUILD_CACHE = {}
LAST_RESULTS = None  # BassKernelResults of the most recent run (for profiling)

POOL_MUL_TILES = 0   # of the 8 tiles per chunk, how many multiply on GPSIMD
DVE_EVAC = set()     # evac unit indices (of 64) routed to DVE instead of ACT


def _build_nc():
    nc = bacc.Bacc(
        "TRN2", target_bir_lowering=False, debug=False, num_devices=N_CORES
    )
    X_d = nc.declare_dram_parameter("X", [NPOS, C_IN], BF16, isOutput=False)
    PT_d = nc.declare_dram_parameter("PT", [P_DIM, NPOS], BF16, isOutput=False)
    Wk_d = nc.declare_dram_parameter("Wk", [P_DIM, Q], BF16, isOutput=False)
    out_d = nc.declare_dram_parameter("out", [NPOS, C_OUT], BF16, isOutput=True)

    relu = mybir.ActivationFunctionType.Relu
    mult = mybir.AluOpType.mult
    add = mybir.AluOpType.add

    ev_idx = [0]

    with ExitStack() as ctx:
        tc = ctx.enter_context(tile.TileContext(nc))
        wkp = ctx.enter_context(tc.tile_pool(name="wk", bufs=1))
        xp = ctx.enter_context(tc.tile_pool(name="x", bufs=3))
        pp = ctx.enter_context(tc.tile_pool(name="pT", bufs=3))
        apool = ctx.enter_context(tc.tile_pool(name="apsum", bufs=2, space="PSUM"))
        wp = ctx.enter_context(tc.tile_pool(name="w", bufs=3))
        mp = ctx.enter_context(tc.tile_pool(name="m", bufs=3))
        t1p = ctx.enter_context(tc.tile_pool(name="t1", bufs=2))
        t2p = ctx.enter_context(tc.tile_pool(name="t2", bufs=2))
        t3p = ctx.enter_context(tc.tile_pool(name="t3", bufs=2))
        t4p = ctx.enter_context(tc.tile_pool(name="t4", bufs=2))
        op = ctx.enter_context(tc.tile_pool(name="o", bufs=3))

        wk_t = wkp.tile([P_DIM, Q], BF16)
        nc.sync.dma_start(out=wk_t[:], in_=Wk_d[:])

        def evac(dst, src):
            """PSUM f32 -> SBUF bf16 with relu, mostly ACT."""
            if ev_idx[0] % (N_TILES // 2) in DVE_EVAC:
                nc.vector.tensor_scalar_max(dst, src, 0.0)
            else:
                nc.scalar.activation(dst, src, relu)
            ev_idx[0] += 1

        post = []        # [(due_chunk, closure)] deferred emissions
        step = [0]

        def flush_post():
            rest = []
            for due, f in post:
                if due <= step[0]:
                    f()
                else:
                    rest.append((due, f))
            post[:] = rest

        PAIR = 2   # tiles per PSUM tile (4 banks; 2 PSUM tiles in flight)
        # leading chunks are small so DVE starts working sooner
        sizes = [2, 2, 4] + [CHUNK] * ((N_TILES - 8) // CHUNK)
        assert sum(sizes) == N_TILES
        tile0 = 0
        for ch, c_t in enumerate(sizes):
            step[0] = ch
            flush_post()
            x_c = xp.tile([TILE_P, c_t, C_IN], BF16)
            nc.sync.dma_start(
                out=x_c[:],
                in_=X_d[tile0 * TILE_P:(tile0 + c_t) * TILE_P, :].rearrange(
                    "(a p) i -> p a i", p=TILE_P
                ),
            )
            pT_c = pp.tile([P_DIM, c_t * TILE_P], BF16)
            nc.sync.dma_start(
                out=pT_c[:],
                in_=PT_d[:, tile0 * TILE_P:(tile0 + c_t) * TILE_P],
            )

            w_c = wp.tile([TILE_P, c_t, Q], BF16)
            for h in range(c_t // PAIR):
                a_t = apool.tile([TILE_P, PAIR, Q], F32)
                for j in range(PAIR):
                    lhsT = pT_c[:, bass.ts(h * PAIR + j, TILE_P)]
                    nc.tensor.matmul(
                        a_t[:, j, 0:512], lhsT=lhsT, rhs=wk_t[:, 0:512],
                        start=True, stop=True,
                    )
                    nc.tensor.matmul(
                        a_t[:, j, 512:1024], lhsT=lhsT,
                        rhs=wk_t[:, 512:1024], start=True, stop=True,
                    )
                evac(w_c[:, bass.ts(h, PAIR), :], a_t[:])

            # m[p, j, o, i] = w[p, j, o, i] * x[p, j, i]  (DVE, 2x bf16)
            m_t = mp.tile([TILE_P, c_t, Q], BF16)
            w4 = w_c[:].rearrange("p j (o i) -> p j o i", o=C_OUT)
            m4 = m_t[:].rearrange("p j (o i) -> p j o i", o=C_OUT)
            x4 = x_c[:].unsqueeze(2).broadcast_to(
                [TILE_P, c_t, C_OUT, C_IN]
            )
            nc.vector.tensor_tensor(out=m4, in0=w4, in1=x4, op=mult)

            def emit_tree(m4=m4, c_t=c_t, tile0=tile0):
                # 5-stage halving tree over i (innermost 32), all on DVE
                t1 = t1p.tile([TILE_P, c_t, C_OUT, 16], BF16)
                nc.vector.tensor_tensor(
                    out=t1[:], in0=m4[:, :, :, 0:16], in1=m4[:, :, :, 16:32],
                    op=add,
                )
                t2 = t2p.tile([TILE_P, c_t, C_OUT, 8], BF16)
                nc.vector.tensor_tensor(
                    out=t2[:], in0=t1[:, :, :, 0:8], in1=t1[:, :, :, 8:16],
                    op=add,
                )
                t3 = t3p.tile([TILE_P, c_t, C_OUT, 4], BF16)
                nc.vector.tensor_tensor(
                    out=t3[:], in0=t2[:, :, :, 0:4], in1=t2[:, :, :, 4:8],
                    op=add,
                )
                t4 = t4p.tile([TILE_P, c_t, C_OUT, 2], BF16)
                nc.vector.tensor_tensor(
                    out=t4[:], in0=t3[:, :, :, 0:2], in1=t3[:, :, :, 2:4],
                    op=add,
                )
                o_c = op.tile([TILE_P, c_t, C_OUT], BF16)
                nc.vector.tensor_tensor(
                    out=o_c[:].unsqueeze(3), in0=t4[:, :, :, 0:1],
                    in1=t4[:, :, :, 1:2], op=add,
                )
                post.append((step[0] + 1, lambda: nc.sync.dma_start(
                    out=out_d[
                        tile0 * TILE_P:(tile0 + c_t) * TILE_P, :
                    ].rearrange("(a p) i -> p a i", p=TILE_P),
                    in_=o_c[:],
                )))

            # defer the tree one chunk to keep the DVE queue deep
            post.append((ch + 1, emit_tree))
            tile0 += c_t

        while post:
            step[0] += 1
            flush_post()

    nc.finalize()
    return nc


def _get_nc():
    key = "v12"
    if key not in _BUILD_CACHE:
        _BUILD_CACHE[key] = _build_nc()
    return _BUILD_CACHE[key]


def kernel(X, P, Wk):
    global LAST_RESULTS
    X = np.asarray(X, dtype=np.float32)
    P = np.asarray(P, dtype=np.float32)
    Wk = np.asarray(Wk, dtype=np.float32)
    bf16 = ml_dtypes.bfloat16

    # Host-side prep (free): shard, transpose P, permute Wk columns so the
    # device-side layout is q = o*32+i; cast matmul operands to bf16.
    WkP = np.ascontiguousarray(
        Wk.reshape(P_DIM, C_IN, C_OUT).transpose(0, 2, 1).reshape(P_DIM, Q)
    ).astype(bf16)
    in_maps = []
    for c in range(N_CORES):
        Xc = np.ascontiguousarray(
            X[c * B_SH:(c + 1) * B_SH].reshape(NPOS, C_IN)
        ).astype(bf16)
        PTc = np.ascontiguousarray(
            P[c * B_SH:(c + 1) * B_SH].reshape(NPOS, P_DIM).T
        ).astype(bf16)
        in_maps.append({"X": Xc, "PT": PTc, "Wk": WkP})

    nc = _get_nc()
    trace = os.environ.get("BASS_PROFILE", "0") == "1"
    kw = {}
    if os.environ.get("BASS_TMPDIR"):
        kw["tmpdir"] = os.environ["BASS_TMPDIR"]
    res = run_bass_kernel_spmd(
        nc, in_maps, list(range(N_CORES)), trace=trace, **kw
    )
    LAST_RESULTS = res

    out = np.empty((B, N, C_OUT), dtype=np.float32)
    for c in range(N_CORES):
        out[c * B_SH:(c + 1) * B_SH] = (
            np.asarray(res.results[c]["out"])
            .astype(np.float32)
            .reshape(B_SH, N, C_OUT)
        )
    return out
